# revision 1
# baseline (speedup 1.0000x reference)
"""Channel (instance) normalization on 8 Trainium2 NeuronCores.

Problem: x [1, 256, 512, 512] f32; per-channel mean / unbiased variance over
the spatial dims; out = (x - mu) / sqrt(var + eps) + beta.  gamma is unused
(reference 'BN' mode).

Sharding: channels 256 -> 32 per core (stats are per-channel, so cores are
fully independent).  Per core, each channel's 512x512 spatial plane is laid
out as one SBUF tile [128 partitions, 2048], processed in groups of 8
channels so the data is read from HBM exactly once: load group -> per-channel
partial sums (free-dim reduce on DVE, square+accumulate on ACT) -> cross-
partition totals via a ones-vector matmul on the PE -> per-group scalar math
-> broadcast of scale/shift to all partitions via a K=1 matmul -> in-place
normalize (one DVE tensor_scalar) -> store.  HBM traffic is the roofline
minimum: 32 MB in + 32 MB out per core.
"""
import numpy as np

import concourse.bass as bass
import concourse.tile as tile
from concourse import mybir
from concourse.bass_utils import run_bass_kernel_spmd

EPS = 1e-5
C, H, W = 256, 512, 512
NCORES = 8
CPC = C // NCORES          # channels per core = 32
GRP = 8                    # channels per stats group
P = 128                    # SBUF partitions
FREE = H * W // P          # 2048 elements per partition per channel
N = H * W                  # elements per channel
f32 = mybir.dt.float32

_MAX_WAITS = 1


def _split_multi_waits(nc):
    """This toolchain's walrus build rejects instructions carrying more than
    one sync wait.  Move extra waits onto same-engine NoOps inserted directly
    before the offending instruction (engines execute their stream in order,
    so this is equivalent)."""
    uid = 0
    for fn in nc.m.functions:
        for bb in fn.blocks:
            out = []
            changed = False
            for inst in bb.instructions:
                si = inst.sync_info
                if si is not None and len(si.on_wait) > _MAX_WAITS:
                    waits = list(si.on_wait)
                    extra, keep = waits[:-_MAX_WAITS], waits[-_MAX_WAITS:]
                    for w in extra:
                        nop = mybir.InstNoOp(name=f"WSNOP-{uid}")
                        uid += 1
                        nop.engine = inst.engine
                        nop.sync_info = mybir.SyncInfo(on_wait=[w], on_update=[])
                        out.append(nop)
                    inst.sync_info = mybir.SyncInfo(
                        on_wait=keep, on_update=list(si.on_update))
                    changed = True
                out.append(inst)
            if changed:
                bb.instructions = out


def _build():
    from contextlib import ExitStack

    nc = bass.Bass()
    x_in = nc.dram_tensor("x", [CPC, H, W], f32, kind="ExternalInput")
    beta_in = nc.dram_tensor("beta", [CPC], f32, kind="ExternalInput")
    y_out = nc.dram_tensor("y", [CPC, H, W], f32, kind="ExternalOutput")

    with tile.TileContext(nc) as tc, ExitStack() as ctx:
        # 2 groups of 8 x 1MB channel tiles live at once (load g+1 while g
        # computes/stores): 16 MB of SBUF.
        xpool = ctx.enter_context(tc.tile_pool(name="xdata", bufs=2 * GRP))
        sqpool = ctx.enter_context(tc.tile_pool(name="sq", bufs=2))
        spool = ctx.enter_context(tc.tile_pool(name="stats", bufs=4))
        pspool = ctx.enter_context(tc.tile_pool(name="ps", bufs=4, space="PSUM"))
        singles = ctx.enter_context(tc.tile_pool(name="singles", bufs=1))

        ones_col = singles.tile([P, 1], f32)   # lhsT for partition-sum matmul
        nc.vector.memset(ones_col, 1.0)
        ones_row = singles.tile([1, P], f32)   # lhsT for the broadcast matmul
        nc.vector.memset(ones_row, 1.0)

        for g in range(CPC // GRP):
            stats = spool.tile([P, 2 * GRP], f32, tag="stats")
            tiles = []
            for i in range(GRP):
                c = g * GRP + i
                t = xpool.tile([P, FREE], f32, tag="xdata")
                nc.sync.dma_start(
                    out=t, in_=x_in[c].rearrange("(p a) w -> p (a w)", p=P))
                tiles.append(t)
                nc.vector.tensor_reduce(
                    out=stats[:, 2 * i:2 * i + 1], in_=t,
                    axis=mybir.AxisListType.X, op=mybir.AluOpType.add)
                sq = sqpool.tile([P, FREE], f32, tag="sq")
                nc.scalar.activation(
                    out=sq, in_=t, func=mybir.ActivationFunctionType.Square,
                    accum_out=stats[:, 2 * i + 1:2 * i + 2])

            # cross-partition totals: [1, 2G] = ones^T @ stats
            tot = pspool.tile([1, 2 * GRP], f32, tag="tot")
            nc.tensor.matmul(out=tot, lhsT=ones_col, rhs=stats,
                             start=True, stop=True)
            totv = tot.rearrange("p (c two) -> p c two", two=2)
            S1, S2 = totv[:, :, 0], totv[:, :, 1]

            beta_row = spool.tile([1, GRP], f32, tag="betar")
            nc.sync.dma_start(out=beta_row,
                              in_=beta_in[g * GRP:(g + 1) * GRP][None, :])

            ab = spool.tile([1, 2 * GRP], f32, tag="ab")  # [A | B]
            A_row, B_row = ab[:, 0:GRP], ab[:, GRP:2 * GRP]
            mu = spool.tile([1, GRP], f32, tag="mu")
            var = spool.tile([1, GRP], f32, tag="var")
            nc.vector.tensor_scalar_mul(out=mu, in0=S1, scalar1=1.0 / N)
            nc.vector.tensor_scalar_mul(out=var, in0=S2, scalar1=1.0 / N)
            nc.vector.tensor_tensor(out=A_row, in0=mu, in1=mu,
                                    op=mybir.AluOpType.mult)
            nc.vector.tensor_tensor(out=var, in0=var, in1=A_row,
                                    op=mybir.AluOpType.subtract)
            # unbiased variance + eps in one op: var*(N/(N-1)) + eps
            nc.vector.tensor_scalar(out=var, in0=var,
                                    scalar1=float(N) / (N - 1), scalar2=EPS,
                                    op0=mybir.AluOpType.mult,
                                    op1=mybir.AluOpType.add)
            nc.scalar.activation(out=var, in_=var,
                                 func=mybir.ActivationFunctionType.Sqrt)
            nc.vector.reciprocal(out=A_row, in_=var)          # A = rstd
            nc.vector.tensor_tensor(out=var, in0=mu, in1=A_row,
                                    op=mybir.AluOpType.mult)
            nc.vector.tensor_tensor(out=B_row, in0=beta_row, in1=var,
                                    op=mybir.AluOpType.subtract)  # B = beta-mu*A

            # broadcast [1, 2G] -> [P, 2G] via K=1 matmul; copy into SBUF so
            # it can feed tensor_scalar's per-partition scalar operands.
            abbc_ps = pspool.tile([P, 2 * GRP], f32, tag="abps")
            nc.tensor.matmul(out=abbc_ps, lhsT=ones_row, rhs=ab,
                             start=True, stop=True)
            AB = spool.tile([P, 2 * GRP], f32, tag="absb")
            nc.vector.tensor_copy(out=AB, in_=abbc_ps)

            for i in range(GRP):
                c = g * GRP + i
                t = tiles[i]
                nc.vector.tensor_scalar(
                    out=t, in0=t, scalar1=AB[:, i:i + 1],
                    scalar2=AB[:, GRP + i:GRP + i + 1],
                    op0=mybir.AluOpType.mult, op1=mybir.AluOpType.add)
                nc.sync.dma_start(
                    out=y_out[c].rearrange("(p a) w -> p (a w)", p=P), in_=t)

    _split_multi_waits(nc)
    return nc


_NC = None


def _get_nc():
    global _NC
    if _NC is None:
        _NC = _build()
    return _NC


def kernel(x, gamma, beta):
    x = np.asarray(x)
    beta = np.asarray(beta).astype(np.float32, copy=False)
    assert x.shape == (1, C, H, W), x.shape
    nc = _get_nc()
    in_maps = [
        {
            "x": np.ascontiguousarray(x[0, i * CPC:(i + 1) * CPC]),
            "beta": np.ascontiguousarray(beta[i * CPC:(i + 1) * CPC]),
        }
        for i in range(NCORES)
    ]
    res = run_bass_kernel_spmd(nc, in_maps, list(range(NCORES)))
    y = np.concatenate([res.results[i]["y"] for i in range(NCORES)], axis=0)
    return y.reshape(1, C, H, W).astype(np.float32, copy=False)


# revision 2
# speedup vs baseline: 17.6981x; 17.6981x over previous
"""Channel (instance) normalization on 8 Trainium NeuronCores.

Problem: x [1, 256, 512, 512] f32; per-channel mean / unbiased (ddof=1)
variance over the spatial dims; out = (x - mu) / sqrt(var + eps) + beta.
gamma is unused (reference 'BN' mode).

Sharding: 256 channels -> 32 per core; per-channel stats are independent so
the 8 cores never communicate.  Per core, each channel's 512x512 plane is one
SBUF tile [128 partitions x 2048] (8 KB contiguous per partition), processed
in groups of 8 channels so HBM traffic is the minimum 32 MB in + 32 MB out:

  loads(g) -> [normalize+store(g-1)] -> stats(g) -> scale/shift(g) -> ...

Per-channel S1 partials via DVE free-dim reduce, S2 partials via one ACT
Square pass with the free-dim accumulator (squares land in PSUM, never SBUF).
Cross-partition totals AND their broadcast to all 128 partitions happen in a
single PE matmul with an all-ones [128,128] stationary matrix.  The scale
A = rstd and shift B = beta - mu*rstd are then computed in broadcast form,
and each channel is normalized in place by one DVE tensor_scalar
(x*A + B) before being stored.  The emission order software-pipelines groups
(loads of group g+1 are issued before normalize/stores of group g) so the
DMA queues never sit behind compute waits; measured on-device rate is ~95%
of the pure HBM-memcpy roofline for the same traffic.
"""
import numpy as np
from contextlib import ExitStack

import concourse.bass as bass
import concourse.tile as tile
from concourse import mybir
from concourse.bass_utils import run_bass_kernel_spmd

EPS = 1e-5
C, H, W = 256, 512, 512
NCORES = 8
CPC = C // NCORES          # channels per core = 32
GRP = 8                    # channels per stats group
P = 128                    # SBUF partitions
FREE = H * W // P          # 2048 elements per partition per channel
N = H * W                  # elements per channel
f32 = mybir.dt.float32

_MAX_WAITS = 1


def _split_multi_waits(nc):
    """This toolchain's walrus build rejects instructions carrying more than
    one sync wait.  Move extra waits onto same-engine NoOps inserted directly
    before the offending instruction (engines execute their stream in order,
    so waiting on the preceding NoOps is equivalent)."""
    uid = 0
    for fn in nc.m.functions:
        for bb in fn.blocks:
            out = []
            changed = False
            for inst in bb.instructions:
                si = inst.sync_info
                if si is not None and len(si.on_wait) > _MAX_WAITS:
                    waits = list(si.on_wait)
                    extra, keep = waits[:-_MAX_WAITS], waits[-_MAX_WAITS:]
                    for w in extra:
                        nop = mybir.InstNoOp(name=f"WSNOP-{uid}")
                        uid += 1
                        nop.engine = inst.engine
                        nop.sync_info = mybir.SyncInfo(on_wait=[w], on_update=[])
                        out.append(nop)
                    inst.sync_info = mybir.SyncInfo(
                        on_wait=keep, on_update=list(si.on_update))
                    changed = True
                out.append(inst)
            if changed:
                bb.instructions = out


def _build():
    nc = bass.Bass()
    x_in = nc.dram_tensor("x", [CPC, H, W], f32, kind="ExternalInput")
    beta_in = nc.dram_tensor("beta", [CPC], f32, kind="ExternalInput")
    y_out = nc.dram_tensor("y", [CPC, H, W], f32, kind="ExternalOutput")
    xf = x_in[:].rearrange("c (p a) w -> p c (a w)", p=P)
    yf = y_out[:].rearrange("c (p a) w -> p c (a w)", p=P)

    with tile.TileContext(nc) as tc, ExitStack() as ctx:
        # 2 groups of 8 x 1MB channel tiles live at once: 16 MB SBUF.
        xpool = ctx.enter_context(tc.tile_pool(name="xdata", bufs=2 * GRP))
        # PSUM: squares scratch [128,2048]f32 = 4 banks (bufs=1) +
        # broadcast-totals [128,16] (1 bank x 2).
        pspool = ctx.enter_context(tc.tile_pool(name="ps", bufs=2, space="PSUM"))
        sqps = ctx.enter_context(tc.tile_pool(name="sqps", bufs=1, space="PSUM"))
        spool = ctx.enter_context(tc.tile_pool(name="stats", bufs=4))
        singles = ctx.enter_context(tc.tile_pool(name="singles", bufs=1))

        ones_sq = singles.tile([P, P], f32)
        nc.vector.memset(ones_sq, 1.0)
        beta_bc = singles.tile([P, CPC], f32)
        b_ap = beta_in[:]
        nc.sync.dma_start(out=beta_bc, in_=bass.AP(
            tensor=b_ap.tensor, offset=b_ap.offset,
            ap=[[0, P]] + list(b_ap.ap)))

        def do_loads(g):
            tiles = []
            for i in range(GRP):
                t = xpool.tile([P, FREE], f32, tag="xdata")
                nc.sync.dma_start(out=t, in_=xf[:, g * GRP + i])
                tiles.append(t)
            return tiles

        def do_stats(g, tiles):
            stats = spool.tile([P, 2 * GRP], f32, tag="stats")
            for i in range(GRP):
                nc.vector.tensor_reduce(
                    out=stats[:, 2 * i:2 * i + 1], in_=tiles[i],
                    axis=mybir.AxisListType.X, op=mybir.AluOpType.add)
            for i in range(GRP):
                sq = sqps.tile([P, FREE], f32, tag="sq")
                nc.scalar.activation(
                    out=sq, in_=tiles[i],
                    func=mybir.ActivationFunctionType.Square,
                    accum_out=stats[:, 2 * i + 1:2 * i + 2])
            # cross-partition totals, broadcast to all partitions in one
            # matmul: tot[m, j] = sum_p stats[p, j] for every row m
            tot = pspool.tile([P, 2 * GRP], f32, tag="tot")
            nc.tensor.matmul(out=tot, lhsT=ones_sq, rhs=stats,
                             start=True, stop=True)
            totv = tot.rearrange("p (c two) -> p c two", two=2)
            S1, S2 = totv[:, :, 0], totv[:, :, 1]

            AB = spool.tile([P, 2 * GRP], f32, tag="ab")
            A, B = AB[:, 0:GRP], AB[:, GRP:2 * GRP]
            mu = spool.tile([P, GRP], f32, tag="mu")
            var = spool.tile([P, GRP], f32, tag="var")
            nc.vector.tensor_scalar_mul(out=mu, in0=S1, scalar1=1.0 / N)
            nc.vector.tensor_scalar_mul(out=var, in0=S2, scalar1=1.0 / N)
            nc.vector.tensor_tensor(out=A, in0=mu, in1=mu,
                                    op=mybir.AluOpType.mult)
            nc.vector.tensor_tensor(out=var, in0=var, in1=A,
                                    op=mybir.AluOpType.subtract)
            # unbiased variance + eps in one op: var*(N/(N-1)) + eps
            nc.vector.tensor_scalar(out=var, in0=var,
                                    scalar1=float(N) / (N - 1), scalar2=EPS,
                                    op0=mybir.AluOpType.mult,
                                    op1=mybir.AluOpType.add)
            nc.scalar.activation(out=var, in_=var,
                                 func=mybir.ActivationFunctionType.Sqrt)
            nc.vector.reciprocal(out=A, in_=var)              # A = rstd
            nc.vector.tensor_tensor(out=var, in0=mu, in1=A,
                                    op=mybir.AluOpType.mult)
            nc.vector.tensor_tensor(out=B,
                                    in0=beta_bc[:, g * GRP:(g + 1) * GRP],
                                    in1=var, op=mybir.AluOpType.subtract)
            return AB

        def do_norm_store(g, tiles, AB):
            for i in range(GRP):
                nc.vector.tensor_scalar(
                    out=tiles[i], in0=tiles[i], scalar1=AB[:, i:i + 1],
                    scalar2=AB[:, GRP + i:GRP + i + 1],
                    op0=mybir.AluOpType.mult, op1=mybir.AluOpType.add)
            for i in range(GRP):
                nc.sync.dma_start(out=yf[:, g * GRP + i], in_=tiles[i])

        prev = None
        for g in range(CPC // GRP):
            tiles = do_loads(g)
            if prev is not None:
                do_norm_store(*prev)
            AB = do_stats(g, tiles)
            prev = (g, tiles, AB)
        do_norm_store(*prev)

    _split_multi_waits(nc)
    return nc


_NC = None


def _get_nc():
    global _NC
    if _NC is None:
        _NC = _build()
    return _NC


def kernel(x, gamma, beta):
    x = np.asarray(x)
    beta = np.asarray(beta).astype(np.float32, copy=False)
    assert x.shape == (1, C, H, W), x.shape
    nc = _get_nc()
    in_maps = [
        {
            "x": np.ascontiguousarray(x[0, i * CPC:(i + 1) * CPC]),
            "beta": np.ascontiguousarray(beta[i * CPC:(i + 1) * CPC]),
        }
        for i in range(NCORES)
    ]
    res = run_bass_kernel_spmd(nc, in_maps, list(range(NCORES)))
    y = np.concatenate([res.results[i]["y"] for i in range(NCORES)], axis=0)
    return y.reshape(1, C, H, W).astype(np.float32, copy=False)
